# revision 36
# baseline (speedup 1.0000x reference)
"""BehaviorMoE Trainium2 kernel (8 NeuronCores, SPMD data-parallel over sorted tokens).

Contract: kernel(**inputs) takes FULL inputs as returned by setup_inputs() and
returns the FULL [8192, 1024] float32 output.

Final design (~132us vs 197us baseline, rel err 8.9e-4):
  - Host: sort tokens by behavior id into 8 single-behavior chunks of M=896
    (7x128 tiles). b==0 tokens need no expert compute; a few are used as
    masked filler inside partial tiles (their device output x+beta is still
    correct), the rest get out = x + beta on host. This drops PE work 12.5%
    vs the 1024-token packing.
  - Everything on the PE is fp16 (x, expert weights, gate weights, biases):
    same 1 cycle/row as f32r/bf16, half the DMA, and enough mantissa that
    the softmax-amplified gate-logit error stays ~1e-2 absolute (bf16
    logits would be unusable). PSUM/accumulation stays f32.
  - Device: gate logits k-paced off the 1.8MB x DMA with PE warm-up
    dummies; the per-tile softmax/bias-combine chain is rotated one tile
    against the first expert pass so its latency hides under expert
    matmuls. Expert stream (e, feature-half, tile) with ping-pong SBUF
    accumulators via DVE scalar_tensor_tensor; weight tiles stream through
    a 48-buffer pool in consumption order.
  - The last three passes (e2c1, e3c0, e3c1) are merged into one per-tile
    sweep so each tile's LN + residual finalize (stats on DVE, scale/bias
    via ACT with fused Abs_reciprocal_sqrt, residual adds on GpSimd,
    per-half output DMA) drains inside the matmul stream; the final tile
    uses DVE for the residual instead, since the drain is latency-bound
    and DVE is idle by then.
"""

import os
import sys

import numpy as np
import ml_dtypes

for _p in ("/opt/trn_rl_repo", "/root/.axon_site/_ro/trn_rl_repo"):
    if os.path.isdir(_p) and _p not in sys.path:
        sys.path.append(_p)

from contextlib import ExitStack

from concourse import bacc, bass, masks, mybir, tile
from concourse.bass_utils import run_bass_kernel_spmd

F32 = mybir.dt.float32
F32R = mybir.dt.float32r
BF16 = mybir.dt.bfloat16
F16 = mybir.dt.float16
AX = mybir.AxisListType
ALU = mybir.AluOpType
ACTF = mybir.ActivationFunctionType

D = 1024            # model dim
N = 8192            # tokens
NB = 4              # behaviors
NESH = 3            # shared experts
NE = 4              # experts per behavior (3 shared + 1 specific)
EPS = 1e-5
NCORES = 8
KT = D // 128       # k tiles (contraction)
FH = 512            # feature half-tile (psum bank width in f32)
M_FULL = N // NCORES       # 1024: fallback packing (all tokens placed)
M_SKIP = 896               # 7 tiles: b==0 tokens mostly skipped

BF = ml_dtypes.bfloat16
HF = np.float16


def _build_program(
    m_tok: int, trivial_affine: bool,
    merged_tail: bool = True, wbufs: int = 48,
) -> bass.Bass:
    IT = m_tok // 128
    H = m_tok // 2          # token half width for gate-logit psum
    nc = bacc.Bacc()

    xt_d = nc.declare_dram_parameter("xt", [KT, 128, m_tok], F16, isOutput=False)
    xtok_d = nc.declare_dram_parameter("xtok", [m_tok, D], F32, isOutput=False)
    wt_d = nc.declare_dram_parameter("wt", [NE, 2, KT, 128, FH], F16, isOutput=False)
    wg_d = nc.declare_dram_parameter("wg", [128, KT * 128], F16, isOutput=False)
    ball_d = nc.declare_dram_parameter("ball", [128, D], F16, isOutput=False)
    mask_d = nc.declare_dram_parameter("mask", [128, IT], F32, isOutput=False)
    if not trivial_affine:
        gam_d = nc.declare_dram_parameter("gam", [128, D], F32, isOutput=False)
        bet_d = nc.declare_dram_parameter("bet", [128, D], F32, isOutput=False)
    out_d = nc.declare_dram_parameter("out", [m_tok, D], F32, isOutput=True)

    with tile.TileContext(nc) as tc, ExitStack() as ctx:
        const = ctx.enter_context(tc.tile_pool(name="const", bufs=1))
        xtp = ctx.enter_context(tc.tile_pool(name="xt", bufs=KT))
        wpool = ctx.enter_context(tc.tile_pool(name="w", bufs=wbufs))
        selp = ctx.enter_context(tc.tile_pool(name="sel", bufs=2 * IT))
        xtokp = ctx.enter_context(tc.tile_pool(name="xtok", bufs=5))
        outp = ctx.enter_context(tc.tile_pool(name="outp", bufs=3))
        scrp = ctx.enter_context(tc.tile_pool(name="scr", bufs=6))
        gatep = ctx.enter_context(tc.tile_pool(name="gate", bufs=IT))
        gbp = ctx.enter_context(tc.tile_pool(name="gb", bufs=IT))
        gtpp = ctx.enter_context(tc.tile_pool(name="gtpp", bufs=4))
        smallp = ctx.enter_context(tc.tile_pool(name="small", bufs=40))
        zpool = ctx.enter_context(tc.tile_pool(name="z", bufs=4, space="PSUM"))
        bppool = ctx.enter_context(tc.tile_pool(name="bp", bufs=2, space="PSUM"))
        pspool = ctx.enter_context(tc.tile_pool(name="ps", bufs=2, space="PSUM"))

        # ---- PE warm-up source (bf16 zeros; dummies use 256-row streams) ----
        zsrc0 = const.tile([128, FH], F32, tag="zsrc0")
        nc.gpsimd.memset(zsrc0[:], 0.0)
        zsrcb = const.tile([128, FH], F16, tag="zsrcb")
        nc.vector.tensor_copy(zsrcb[:], zsrc0[:])
        eps_sb = const.tile([128, 1], F32, tag="eps")
        nc.gpsimd.memset(eps_sb[:], EPS)

        dummy_state = {"n": 0}

        def dummies(n):
            """n cheap bf16 PE filler matmuls (zero @ zero) to hold p-state."""
            dt_ = zpool.tile([128, 256], F32, tag="z", name=f"dps{dummy_state['n']}")
            dummy_state["n"] += 1
            for j in range(n):
                nc.tensor.matmul(
                    dt_[:], zsrcb[:, 0:128], zsrcb[:, 0:256],
                    start=(j == 0), stop=(j == n - 1),
                )

        # ---- gate weights first (needed by glT k0); the x stream follows
        #      immediately, and the remaining consts ride behind it (they
        #      are not read until the bias/softmax phase) ----
        wg_sb = const.tile([128, KT * 128], F16, tag="wg")
        nc.sync.dma_start(wg_sb[:], wg_d[:])

        # ---- resident xT k-tiles (f32 for gates) interleaved with the first
        #      expert's weight tiles so the expert stream starts as soon as
        #      the gate phase drains; remaining weights stream after ----
        xT = []
        w_sb = {}

        def w_dma(e, c, k):
            t = wpool.tile([128, FH], F16, tag="w", name=f"w{e}{c}{k}")
            nc.sync.dma_start(t[:], wt_d[e, c, k])
            w_sb[(e, c, k)] = t

        # issue order matches the (e, c, k) consumption order
        for k in range(KT):
            t = xtp.tile([128, m_tok], F16, tag="xt")
            nc.sync.dma_start(t[:], xt_d[k])
            xT.append(t)
        ball_sb = const.tile([128, D], F16, tag="ball")
        nc.sync.dma_start(ball_sb[:], ball_d[:])
        mask_sb = const.tile([128, IT], F32, tag="mask")
        nc.sync.dma_start(mask_sb[:], mask_d[:])
        if not trivial_affine:
            gam_sb = const.tile([128, D], F32, tag="gam")
            nc.sync.dma_start(gam_sb[:], gam_d[:])
            bet_sb = const.tile([128, D], F32, tag="bet")
            nc.sync.dma_start(bet_sb[:], bet_d[:])
        for e in range(NE):
            for c in (0, 1):
                for k in range(KT):
                    w_dma(e, c, k)

        identity = const.tile([128, 128], F32, tag="ident")
        masks.make_identity(nc, identity[:])
        identR = const.tile([128, 128], F32R, tag="identR")
        nc.vector.tensor_copy(identR[:], identity[:])
        identB = const.tile([128, 128], F16, tag="identB")
        nc.vector.tensor_copy(identB[:], identity[:])

        xB = xT  # fp16 x feeds both the gate matmul and the expert stream

        # ---- accumulators (ping-pong; in-place DVE ops fault) ----
        selA = [selp.tile([128, D], F32, tag="sel", name=f"selA{i}") for i in range(IT)]
        selB = [selp.tile([128, D], F32, tag="sel", name=f"selB{i}") for i in range(IT)]

        gates_t = []
        gatesb_t = []

        def stt_acc(e, c, i, dst_l, src_l):
            cs = slice(c * FH, (c + 1) * FH)
            isl = slice(i * 128, (i + 1) * 128)
            zt = zpool.tile([128, FH], F32, tag="z")
            for k in range(KT):
                nc.tensor.matmul(
                    zt[:], xB[k][:, isl], w_sb[(e, c, k)][:],
                    start=(k == 0), stop=(k == KT - 1),
                )
            nc.vector.scalar_tensor_tensor(
                dst_l[i][:, cs], zt[:], gates_t[i][:, e:e + 1],
                src_l[i][:, cs], op0=ALU.mult, op1=ALU.add,
            )

        # ---- gate logits glT[4, tok], k-paced by the xT DMAs ----
        dummies(12)
        glT_ps = {}
        for h in (0, 1):
            glT_ps[h] = pspool.tile([128, H], F32, tag="ps", name=f"glTps{h}")
        for k in range(KT):
            for h in (0, 1):
                nc.tensor.matmul(
                    glT_ps[h][:], wg_sb[:, k * 128:(k + 1) * 128],
                    xT[k][:, h * H:(h + 1) * H],
                    start=(k == 0), stop=(k == KT - 1),
                )
            if k < KT - 1:
                dummies(3)
        glT_sb = const.tile([NE, m_tok], F32R, tag="glT")
        for h in (0, 1):
            nc.vector.tensor_copy(glT_sb[:, h * H:(h + 1) * H], glT_ps[h][0:NE, :])

        # ---- per token tile: logits transpose (PE), masked softmax, gates
        #      transpose, bias-combine matmuls, then the e0c0 expert group
        #      of the PREVIOUS tile (its matmuls fill the PE while this
        #      tile's softmax runs on DVE/ACT) ----
        for i in range(IT + 1):
            if i < IT:
                glp = pspool.tile([128, NE], F32, tag="ps", name=f"glp{i}")
                nc.tensor.matmul(
                    glp[:], glT_sb[:, i * 128:(i + 1) * 128], identR[0:NE, 0:NE],
                    start=True, stop=True,
                )
                negmax = smallp.tile([128, 1], F32, tag="s1")
                nc.vector.tensor_reduce(
                    negmax[:], glp[:], axis=AX.X, op=ALU.max, negate=True
                )
                exps = smallp.tile([128, NE], F32, tag="s4")
                expsum = smallp.tile([128, 1], F32, tag="s1")
                nc.scalar.activation(
                    exps[:], glp[:], ACTF.Exp,
                    bias=negmax[:], scale=1.0, accum_out=expsum[:],
                )
                rinv = smallp.tile([128, 1], F32, tag="s1")
                nc.vector.reciprocal(rinv[:], expsum[:])
                rm = smallp.tile([128, 1], F32, tag="s1")
                nc.vector.tensor_mul(rm[:], rinv[:], mask_sb[:, i:i + 1])
                gates = gatep.tile([128, NE], F32, tag="g")
                nc.vector.tensor_scalar_mul(gates[:], exps[:], rm[:])
                gates_t.append(gates)
                gb = gbp.tile([128, NE], F16, tag="gb")
                nc.vector.tensor_copy(gb[:], gates[:])
                gatesb_t.append(gb)
            if i > 0:
                stt_acc(0, 0, i - 1, selB, selA)
            if i < IT:
                gtp = pspool.tile([NE, 128], F32, tag="ps", name=f"gtp{i}")
                nc.tensor.matmul(
                    gtp[:], gatesb_t[i][:], identB[:], start=True, stop=True
                )
                gtpsb = gtpp.tile([NE, 128], F16, tag="gtpsb")
                nc.vector.tensor_copy(gtpsb[:], gtp[:])
                for c in (0, 1):
                    bp = bppool.tile([128, FH], F32, tag="bp", name=f"bps{i}{c}")
                    nc.tensor.matmul(
                        bp[:], gtpsb[:], ball_sb[0:NE, c * FH:(c + 1) * FH],
                        start=True, stop=True,
                    )
                    nc.scalar.copy(selA[i][:, c * FH:(c + 1) * FH], bp[:])

        # ---- expert matmul stream: (e, c, i) passes with DVE
        #      scalar_tensor_tensor accumulating gate-weighted outputs in
        #      ping-pong SBUF tiles. The last three passes (e2c1, e3c0,
        #      e3c1) are merged into one per-tile sweep: each tile gets
        #      ~5.1us of PE window for its ~4.3us DVE finalize (stats, LN,
        #      residual), so the tail drains with the stream instead of
        #      after it. Residual: half 0 on DVE (x - mu*r precomputed on
        #      ACT), half 1 on ACT+GpSimd. ----
        xi_t = [None] * IT

        # sel parity: bias in A; e0: A->B, e1: B->A, e2: A->B, e3: B->A
        bn12s = [None] * IT

        def ln_tail(i):
            selF = selA[i]
            xi = xi_t[i]
            bn12 = bn12s[i]
            nc.vector.bn_stats(bn12[:, 6:12], selF[:, FH:D])
            mv = smallp.tile([128, 2], F32, tag="mv")
            nc.vector.bn_aggr(mv[:], bn12[:])
            ri = smallp.tile([128, 1], F32, tag="s1")
            nc.scalar.activation(
                ri[:], mv[:, 1:2], ACTF.Abs_reciprocal_sqrt,
                bias=eps_sb[:], scale=1.0,
            )
            mbi = smallp.tile([128, 1], F32, tag="s1")
            nc.vector.tensor_scalar(
                mbi[:], mv[:, 0:1], ri[:], -1.0,
                op0=ALU.mult, op1=ALU.mult,
            )
            outt = outp.tile([128, D], F32, tag="out")
            if trivial_affine:
                if i == IT - 1:
                    # last tile: the drain chain is latency-bound and DVE is
                    # idle by now — out = sel*r + (x - mu*r) via ACT+DVE
                    # (0.75us STT) instead of the 1.27us GpSimd adds
                    for c in (0, 1):
                        cs = slice(c * FH, (c + 1) * FH)
                        xadj = scrp.tile([128, FH], F32, tag="scr")
                        nc.scalar.activation(
                            xadj[:], xi[:, cs], ACTF.Identity,
                            bias=mbi[:], scale=1.0,
                        )
                        nc.vector.scalar_tensor_tensor(
                            outt[:, cs], selF[:, cs], ri[:], xadj[:],
                            op0=ALU.mult, op1=ALU.add,
                        )
                        nc.sync.dma_start(
                            out_d[i * 128:(i + 1) * 128, cs], outt[:, cs]
                        )
                    return
                # ln = sel*r - mu*r per half on ACT; residual adds on GpSimd;
                # each half's out DMA departs as soon as its add lands
                ln0 = scrp.tile([128, FH], F32, tag="scr")
                nc.scalar.activation(
                    ln0[:], selF[:, 0:FH], ACTF.Identity,
                    bias=mbi[:], scale=ri[:],
                )
                nc.gpsimd.tensor_add(outt[:, 0:FH], ln0[:], xi[:, 0:FH])
                nc.sync.dma_start(
                    out_d[i * 128:(i + 1) * 128, 0:FH], outt[:, 0:FH]
                )
                ln1 = scrp.tile([128, FH], F32, tag="scr")
                nc.scalar.activation(
                    ln1[:], selF[:, FH:D], ACTF.Identity,
                    bias=mbi[:], scale=ri[:],
                )
                nc.gpsimd.tensor_add(outt[:, FH:D], ln1[:], xi[:, FH:D])
                nc.sync.dma_start(
                    out_d[i * 128:(i + 1) * 128, FH:D], outt[:, FH:D]
                )
                return
            else:
                lnb = scrp.tile([128, D], F32, tag="scr2")
                nc.scalar.activation(
                    lnb[:], selF[:], ACTF.Identity,
                    bias=mbi[:], scale=ri[:],
                )
                lng = scrp.tile([128, D], F32, tag="scr2")
                nc.vector.tensor_mul(lng[:], lnb[:], gam_sb[:])
                lnb2 = scrp.tile([128, D], F32, tag="scr2")
                nc.vector.tensor_add(lnb2[:], lng[:], bet_sb[:])
                nc.gpsimd.tensor_add(outt[:, 0:FH], lnb2[:, 0:FH], xi[:, 0:FH])
                nc.gpsimd.tensor_add(outt[:, FH:D], lnb2[:, FH:D], xi[:, FH:D])
            nc.sync.dma_start(out_d[i * 128:(i + 1) * 128, :], outt[:])

        def xi_prefetch(i):
            xi = xtokp.tile([128, D], F32, tag="xtok")
            nc.sync.dma_start(xi[:], xtok_d[i * 128:(i + 1) * 128, :])
            xi_t[i] = xi

        if merged_tail:
            plain_passes = [(0, 1), (1, 0), (1, 1), (2, 0)]
        else:
            plain_passes = [(0, 1), (1, 0), (1, 1), (2, 0), (2, 1), (3, 0)]
        for e, c in plain_passes:
            dst_l = selB if e % 2 == 0 else selA
            src_l = selA if e % 2 == 0 else selB
            for i in range(IT):
                stt_acc(e, c, i, dst_l, src_l)
                if (e, c) == (2, 0) and merged_tail:
                    xi_prefetch(i)
                if (e, c) == (3, 0):
                    xi_prefetch(i)
                    bn12 = smallp.tile([128, 12], F32, tag="bn12")
                    nc.vector.bn_stats(bn12[:, 0:6], selA[i][:, 0:FH])
                    bn12s[i] = bn12

        if merged_tail:
            for i in range(IT):
                stt_acc(2, 1, i, selB, selA)       # e2 half 1
                bn12 = smallp.tile([128, 12], F32, tag="bn12")
                bn12s[i] = bn12
                stt_acc(3, 0, i, selA, selB)       # e3 half 0 (final)
                nc.vector.bn_stats(bn12[:, 0:6], selA[i][:, 0:FH])
                stt_acc(3, 1, i, selA, selB)       # e3 half 1 (final)
                ln_tail(i)
        else:
            for i in range(IT):
                stt_acc(3, 1, i, selA, selB)       # e3 half 1 (final)
                ln_tail(i)

    nc.finalize()
    return nc


_PROGRAM_CACHE: dict = {}


def _get_program(m_tok: int, trivial_affine: bool) -> bass.Bass:
    key = (m_tok, trivial_affine)
    if key not in _PROGRAM_CACHE:
        _PROGRAM_CACHE[key] = _build_program(m_tok, trivial_affine)
    return _PROGRAM_CACHE[key]


def _pack_tokens(b: np.ndarray, m_tok: int, use_all_fill: bool):
    """Partition tokens into 8 chunks of m_tok, each chunk holding tokens of a
    single behavior (1..4) plus masked b==0 filler. Returns (cores, leftover)
    where leftover are b==0 tokens not placed on any core (None on failure)."""
    idx0 = np.flatnonzero(b == 0)
    chunks = []
    for t in range(1, NB + 1):
        idxs = np.flatnonzero(b == t)
        for s in range(0, max(len(idxs), 1), m_tok):
            part = idxs[s:s + m_tok]
            if len(part):
                chunks.append((part, t))
    if len(chunks) > NCORES:
        return None, None
    while len(chunks) < NCORES:
        chunks.append((np.empty((0,), np.int64), 1))
    need_total = sum(m_tok - len(p) for p, _ in chunks)
    if need_total > len(idx0):
        return None, None
    p0 = 0
    cores = []
    for part, t in chunks:
        need = m_tok - len(part)
        fill = idx0[p0:p0 + need]
        p0 += need
        idx = np.concatenate([part.astype(np.int64), fill.astype(np.int64)])
        msk = np.zeros((m_tok,), np.float32)
        msk[:len(part)] = 1.0
        cores.append((idx, msk, t))
    leftover = idx0[p0:]
    if use_all_fill and len(leftover):
        return None, None
    return cores, leftover


def _behavior_tensors(W_sh, b_sh, W_sp, b_sp, w_gates):
    per_t = {}
    W_sh_flat = W_sh.reshape(NESH * D, D)
    for t in range(1, NB + 1):
        Wall = np.concatenate([W_sh_flat, W_sp[t - 1:t].reshape(D, D)], axis=0)
        wT = np.ascontiguousarray(Wall.T)                      # [D, NE*D]
        wt_h = np.ascontiguousarray(
            wT.reshape(KT, 128, NE, 2, FH).transpose(2, 3, 0, 1, 4).astype(HF)
        )                                                      # [e, c, k, 128, FH]
        wg_h = np.zeros((128, KT * 128), HF)
        wg_k = w_gates[t - 1].reshape(KT, 128, NE).transpose(1, 0, 2)  # [128, KT, NE]
        for k in range(KT):
            wg_h[:, k * 128:k * 128 + NE] = wg_k[:, k, :]
        ball_h = np.zeros((128, D), np.float32)
        ball_h[0:NE] = np.stack([b_sh[0], b_sh[1], b_sh[2], b_sp[t - 1]], axis=0)
        per_t[t] = (wt_h, wg_h, np.ascontiguousarray(ball_h.astype(HF)))
    return per_t


def _prepare(x, b_seq, W_sh, b_sh, W_sp, b_sp, w_gates, gamma, beta):
    x = np.ascontiguousarray(np.asarray(x, dtype=np.float32))
    b = np.asarray(b_seq).astype(np.int64).ravel()
    W_sh = np.asarray(W_sh, dtype=np.float32)
    b_sh = np.asarray(b_sh, dtype=np.float32)
    W_sp = np.asarray(W_sp, dtype=np.float32)
    b_sp = np.asarray(b_sp, dtype=np.float32)
    w_gates = np.asarray(w_gates, dtype=np.float32)
    gamma = np.asarray(gamma, dtype=np.float32)
    beta = np.asarray(beta, dtype=np.float32)
    assert x.shape == (N, D) and b.shape == (N,)

    trivial = bool(np.all(gamma == 1.0) and np.all(beta == 0.0))

    m_tok = M_SKIP
    cores, leftover = _pack_tokens(b, M_SKIP, use_all_fill=False)
    if cores is None:
        m_tok = M_FULL
        cores, leftover = _pack_tokens(b, M_FULL, use_all_fill=False)
        if cores is None:
            raise RuntimeError("token packing failed for both chunk sizes")

    per_t = _behavior_tensors(W_sh, b_sh, W_sp, b_sp, w_gates)

    IT = m_tok // 128
    in_maps = []
    for idx, msk, t in cores:
        wt_h, wg_h, ball_h = per_t[t]
        xc = np.ascontiguousarray(x[idx])                      # [M, D]
        xt_h = np.ascontiguousarray(xc.T.astype(HF)).reshape(KT, 128, m_tok)
        m = {
            "xt": xt_h,
            "xtok": xc,
            "wt": wt_h,
            "wg": wg_h,
            "ball": ball_h,
            "mask": np.ascontiguousarray(msk.reshape(IT, 128).T),
        }
        if not trivial:
            m["gam"] = np.ascontiguousarray(np.broadcast_to(gamma, (128, D)))
            m["bet"] = np.ascontiguousarray(np.broadcast_to(beta, (128, D)))
        in_maps.append(m)
    return trivial, m_tok, cores, leftover, (x, beta), in_maps


def kernel_with_results(trace: bool = False, **inputs):
    trivial, m_tok, cores, leftover, (x, beta), in_maps = _prepare(**inputs)
    nc = _get_program(m_tok, trivial)
    res = run_bass_kernel_spmd(
        nc, in_maps, list(range(NCORES)), trace=trace
    )
    out = np.empty((N, D), np.float32)
    for c, (idx, _msk, _t) in enumerate(cores):
        out[idx] = res.results[c]["out"]
    if leftover is not None and len(leftover):
        # b==0 tokens that were not needed as filler: out = x + beta
        out[leftover] = x[leftover] + beta[None, :]
    return out, res


def kernel(**inputs) -> np.ndarray:
    out, _ = kernel_with_results(trace=False, **inputs)
    return out
